# revision 6
# baseline (speedup 1.0000x reference)
"""Bahdanau attention kernel for Trainium2 (8 NeuronCores, data-parallel over batch).

Reference computation (per batch row b):
    pq      = query @ Wq.T                       # (B, AD)
    hidden  = tanh(pq[:, None, :] + processed_memory)   # (B, T, AD)
    e       = einsum('btd,d->bt', hidden, v)     # (B, T)
    e       = where(mask, -1e30, e)
    out     = softmax(e, axis=1)

Sparsity: masked positions (mask==True, ~50% of T) contribute exactly 0 to the
softmax output and denominator (exp(-1e30) underflows to 0), so the host
compacts each batch row to its unmasked columns only (a gather is layout prep,
like the transpose the kernel already requires), padded to a fixed Tc.  The
device streams/tanhs/matmuls ~Tc=2176 columns instead of T=4096 — about half
the HBM traffic and half the ScalarE tanh work (the bottleneck engine:
1 elem/cycle/partition at 1.2 GHz, no fp16 discount).

Device strategy (per core, 8 batches):
  * compacted pm is host-laid-out [b, p, d, t] fp16 so each SBUF partition row
    is one contiguous 2*Tc run (8704B DMA packets — the DMA engines are
    packet-rate limited, fat rows double effective bandwidth vs [b, d*128+p, t]).
    One DMA per batch on the sync queue; constants (Wq/qT/v8/keepc) ride the
    gpsimd queue in parallel so pq never blocks the pm stream.
  * the per-d "+pq" add folds into the ScalarE tanh as a per-partition
    activation bias (free).  A dummy tanh at kernel start pulls the
    ACT_TABLE_LOAD (1.3us) off the critical path.
  * energies accumulate into ONE shared PSUM region [8, Tc] (one 512-wide
    chunk tile per bank): the stationary for batch b is v (x) e_b, a [128, 8]
    one-hot column matrix, so batch b's matmuls land in PSUM row b while other
    batches' matmuls add exact zeros there.  Matmuls trail each tanh
    immediately; PE cost is free-size bound, unchanged by M=8.
  * softmax runs directly on the [8, Tc] layout with no relayout: exp reads
    each PSUM chunk directly (PSUM->SBUF fused), using accum_out to produce
    per-batch row sums for chunks that cannot contain padding
    (col < floor(min_cnt/512)*512); only the tail columns get a host-built
    keep-multiply + reduce on DVE.  Padded output columns are never zeroed —
    the host scatter drops them.  The 1/rowsum scale splits between ScalarE
    (activation Copy with per-partition scale) and DVE so both run in
    parallel.  Out rows are fat 8704B packets.
"""

import sys

if "/opt/trn_rl_repo" not in sys.path:
    sys.path.insert(0, "/opt/trn_rl_repo")

import numpy as np

import concourse.bacc as bacc
import concourse.bass as bass
import concourse.tile as tile
from concourse import mybir
from concourse.bass_utils import run_bass_kernel_spmd

B, T, QD, AD = 64, 4096, 1024, 256
NCORES = 8
BLOC = B // NCORES  # batches per core
KB = QD // 128      # k-blocks for the pq matmul
DB = AD // 128      # d-blocks (partition blocks of AD)
F32 = mybir.dt.float32
F16 = mybir.dt.float16


def build_nc(Tc: int, C0c: int) -> bass.Bass:
    chunks = []
    lo = 0
    while lo < Tc:
        chunks.append((lo, min(512, Tc - lo)))
        lo += 512
    NCH = len(chunks)
    C0 = C0c * 512        # first column that may contain padding
    Mw = Tc - C0          # width of the keep-masked tail

    nc = bacc.Bacc(None, target_bir_lowering=False)

    pm2 = nc.declare_dram_parameter("pm2", [BLOC, 128, DB, Tc], F16, isOutput=False)
    qT = nc.declare_dram_parameter("qT", [128, KB * BLOC], F16, isOutput=False)
    # WqR[p, kb, d] = Wq[d, kb*128 + p]  (host-packed: contiguous 512B rows)
    WqR = nc.declare_dram_parameter("WqR", [128, KB, AD], F16, isOutput=False)
    # v8[p, d*BLOC+b, j] = v[d*128+p] * (j == b): one-hot stationaries that
    # route batch b's energies into PSUM row b
    v8 = nc.declare_dram_parameter("v8", [128, DB * BLOC, 8], F16, isOutput=False)
    # keepc[b, j] = 1.0 iff compacted column C0+j of batch b is real (not pad)
    keepc = nc.declare_dram_parameter("keepc", [BLOC, Mw], F32, isOutput=False)
    out = nc.declare_dram_parameter("out", [BLOC, Tc], F32, isOutput=True)

    Tanh = mybir.ActivationFunctionType.Tanh
    Exp = mybir.ActivationFunctionType.Exp
    Copy = mybir.ActivationFunctionType.Copy

    with tile.TileContext(nc) as tc:
        with (
            tc.tile_pool(name="singles", bufs=1) as singles,
            tc.tile_pool(name="pm", bufs=3) as pm_pool,
            tc.tile_pool(name="hid", bufs=3) as hid_pool,
            tc.tile_pool(name="energy", bufs=1, space="PSUM") as ep_pool,
            tc.tile_pool(name="spsum", bufs=2, space="PSUM") as sp_pool,
        ):
            # ---- act-table warm-up: dummy tanh so the 1.3us ACT_TABLE_LOAD
            # runs during startup DMA instead of gating the first real tanh
            zw = singles.tile([1, 1], F32)
            nc.gpsimd.memset(zw, 0)
            zw2 = singles.tile([1, 1], F32)
            nc.scalar.activation(out=zw2, in_=zw, func=Tanh, bias=zw[0:1, 0:1])

            # ---- constants on the gpsimd queue (parallel with pm stream) ----
            wq_sb = singles.tile([128, KB, AD], F16)
            nc.gpsimd.dma_start(out=wq_sb, in_=WqR[:, :, :])
            qt_sb = singles.tile([128, KB, BLOC], F16)
            nc.gpsimd.dma_start(
                out=qt_sb, in_=qT[:, :].rearrange("p (kb b) -> p kb b", b=BLOC)
            )
            v8_sb = singles.tile([128, DB * BLOC, 8], F16)
            nc.gpsimd.dma_start(out=v8_sb, in_=v8[:, :, :])
            kc_sb = singles.tile([BLOC, Mw], F32)
            nc.gpsimd.dma_start(out=kc_sb, in_=keepc[:, :])

            # ---- pq = Wq @ query.T, laid out [d % 128, dblk, b] ----
            pq_sb = singles.tile([128, DB, BLOC], F32)
            for d in range(DB):
                ppq = sp_pool.tile([128, BLOC], F32, tag="sp")
                for k in range(KB):
                    nc.tensor.matmul(
                        ppq,
                        lhsT=wq_sb[:, k, d * 128 : (d + 1) * 128],
                        rhs=qt_sb[:, k, :],
                        start=(k == 0),
                        stop=(k == KB - 1),
                    )
                nc.vector.tensor_copy(out=pq_sb[:, d, :], in_=ppq)

            # ---- shared energies accumulator: one [8, w] PSUM tile per bank ----
            ep = []
            for ci, (_, w) in enumerate(chunks):
                ep_ci = ep_pool.tile([BLOC, w], F32, tag=f"ep{ci}")
                ep.append(ep_ci)

            # ---- main loop: tanh + one-hot v-reduction ----
            for b in range(BLOC):
                pm_sb = pm_pool.tile([128, DB, Tc], F16)
                nc.sync.dma_start(out=pm_sb, in_=pm2[b])
                h = hid_pool.tile([128, DB, Tc], F16)
                for d in range(DB):
                    nc.scalar.activation(
                        out=h[:, d, :],
                        in_=pm_sb[:, d, :],
                        func=Tanh,
                        bias=pq_sb[:, d, b : b + 1],
                        scale=1.0,
                    )
                    first = b == 0 and d == 0
                    last = b == BLOC - 1 and d == DB - 1
                    for ci, (lo, w) in enumerate(chunks):
                        nc.tensor.matmul(
                            ep[ci],
                            lhsT=v8_sb[:, d * BLOC + b, :],
                            rhs=h[:, d, lo : lo + w],
                            start=first,
                            stop=last,
                            skip_group_check=True,
                        )

            # ---- softmax on [8, Tc]: exp straight out of PSUM ----
            work = singles.tile([BLOC, Tc], F32)
            accs = []
            # masked tail chunks first so their DVE work overlaps later exps
            for ci in range(C0c, NCH):
                lo, w = chunks[ci]
                nc.scalar.activation(out=work[:, lo : lo + w], in_=ep[ci], func=Exp)
            if Mw > 0:
                nc.vector.tensor_mul(work[:, C0:Tc], work[:, C0:Tc], kc_sb)
                macc = singles.tile([BLOC, 1], F32)
                nc.vector.reduce_sum(
                    out=macc, in_=work[:, C0:Tc], axis=mybir.AxisListType.X
                )
                accs.append(macc)
            for ci in range(C0c):
                lo, w = chunks[ci]
                acc = singles.tile([BLOC, 1], F32, name=f"acc{ci}")
                nc.scalar.activation(
                    out=work[:, lo : lo + w], in_=ep[ci], func=Exp, accum_out=acc
                )
                accs.append(acc)
            rsum = singles.tile([BLOC, 1], F32)
            nc.vector.tensor_add(out=rsum, in0=accs[0], in1=accs[1])
            for acc in accs[2:]:
                nc.vector.tensor_add(out=rsum, in0=rsum, in1=acc)
            rinv = singles.tile([BLOC, 1], F32)
            nc.vector.reciprocal(out=rinv, in_=rsum)

            # ---- scale by 1/rowsum, split ScalarE || DVE ----
            ow = singles.tile([BLOC, Tc], F32)
            SP = 1024  # ScalarE takes [0,SP), DVE takes [SP,Tc)
            nc.scalar.activation(
                out=ow[:, 0:SP], in_=work[:, 0:SP], func=Copy, scale=rinv[:, 0:1]
            )
            nc.vector.tensor_scalar_mul(
                out=ow[:, SP:Tc], in0=work[:, SP:Tc], scalar1=rinv[:, 0:1]
            )
            nc.gpsimd.dma_start(out=out[:, :], in_=ow)

    nc.finalize()
    return nc


_CACHE: dict = {}


def _get_nc(key) -> bass.Bass:
    if key not in _CACHE:
        _CACHE[key] = build_nc(*key)
    return _CACHE[key]


def _pick_tc(max_cnt: int) -> int:
    # fixed padded width, multiple of 128; 2176 covers the reference seed
    # (max count 2126) — recomputed per call so any mask works
    return max(2176, -(-(max_cnt + 1) // 128) * 128)


def make_in_maps(query, processed_memory, mask, Wq, v):
    query = np.ascontiguousarray(np.asarray(query, dtype=np.float32))
    pm = np.asarray(processed_memory, dtype=np.float32)
    mask_b = np.asarray(mask).astype(bool)
    Wq = np.asarray(Wq, dtype=np.float32)
    v = np.asarray(v, dtype=np.float32)

    keep = ~mask_b
    keep_idx = [np.flatnonzero(keep[gb]) for gb in range(B)]
    cnts = np.array([len(ix) for ix in keep_idx])
    Tc = _pick_tc(int(cnts.max()))
    nch = -(-Tc // 512)
    C0c = min(int(cnts.min()) // 512, nch - 1)
    C0 = C0c * 512
    Mw = Tc - C0
    key = (Tc, C0c)

    WqR = np.ascontiguousarray(
        Wq.T.reshape(KB, 128, AD).transpose(1, 0, 2).astype(np.float16)
    )
    v8 = np.zeros((128, DB * BLOC, 8), dtype=np.float16)
    for d in range(DB):
        for b in range(BLOC):
            v8[:, d * BLOC + b, b] = v[d * 128 : (d + 1) * 128]

    in_maps = []
    for i in range(NCORES):
        sl = slice(i * BLOC, (i + 1) * BLOC)
        pm2 = np.zeros((BLOC, 128, DB, Tc), dtype=np.float16)
        keepc = np.zeros((BLOC, Mw), dtype=np.float32)
        for b in range(BLOC):
            gb = i * BLOC + b
            c = cnts[gb]
            # [c, AD] -> [AD, c] -> [DB, 128, c] -> [128, DB, c]
            pm2[b, :, :, :c] = (
                pm[gb, keep_idx[gb], :].T.reshape(DB, 128, c).transpose(1, 0, 2)
            )
            keepc[b, : c - C0] = 1.0
        in_maps.append(
            {
                "pm2": pm2,
                "qT": np.ascontiguousarray(
                    query[sl]
                    .T.reshape(KB, 128, BLOC)
                    .transpose(1, 0, 2)
                    .reshape(128, KB * BLOC)
                    .astype(np.float16)
                ),
                "WqR": WqR,
                "v8": v8,
                "keepc": keepc,
            }
        )
    return in_maps, keep_idx, cnts, key


def run_spmd(in_maps, key=(2176, 3), **kwargs):
    return run_bass_kernel_spmd(_get_nc(key), in_maps, list(range(NCORES)), **kwargs)


def kernel(query, processed_memory, mask, Wq, v) -> np.ndarray:
    in_maps, keep_idx, cnts, key = make_in_maps(query, processed_memory, mask, Wq, v)
    res = run_spmd(in_maps, key=key)
    full = np.zeros((B, T), dtype=np.float32)
    for i in range(NCORES):
        outc = np.asarray(res.results[i]["out"], dtype=np.float32)
        for b in range(BLOC):
            gb = i * BLOC + b
            full[gb, keep_idx[gb]] = outc[b, : cnts[gb]]
    return full


# revision 8
# speedup vs baseline: 1.1906x; 1.1906x over previous
"""Bahdanau attention kernel for Trainium2 (8 NeuronCores, data-parallel over batch).

Reference computation (per batch row b):
    pq      = query @ Wq.T                       # (B, AD)
    hidden  = tanh(pq[:, None, :] + processed_memory)   # (B, T, AD)
    e       = einsum('btd,d->bt', hidden, v)     # (B, T)
    e       = where(mask, -1e30, e)
    out     = softmax(e, axis=1)

Sparsity: masked positions (mask==True, ~50% of T) contribute exactly 0 to the
softmax output and denominator (exp(-1e30) underflows to 0), so the host
compacts each batch row to its unmasked columns only (a gather is layout prep,
like the transpose the kernel already requires), padded to a fixed Tc.  The
device streams/tanhs/matmuls ~Tc=2176 columns instead of T=4096 — about half
the HBM traffic and half the ScalarE tanh work (the bottleneck engine:
1 elem/cycle/partition at 1.2 GHz, no fp16 discount).

Device strategy (per core, 8 batches):
  * compacted pm is host-laid-out [b, p, d, t] fp16 so each SBUF partition row
    is one contiguous 2*Tc = 8704B run (the DMA engines are rate-limited to
    ~11 B/ns each; fat contiguous rows keep them at peak).  The pm batches are
    split across BOTH hardware-dynamic DMA queues (sync + gpsimd) because one
    queue alone sustains only ~200 GB/s; gpsimd's DGE has a ~12us launch ramp
    so the first two batches ride the sync queue.
  * padding columns hold pm = -16*sign(v[d]), so every padded energy is
    ~ -sum|v| ~= -12.8 and exp() makes it ~3e-6: no keep-mask, no masked
    reduce, and the host scatter drops padded outputs anyway.
  * the per-d "+pq" add folds into the ScalarE tanh as a per-partition
    activation bias (free).  A dummy tanh at kernel start pulls the 1.3us
    ACT_TABLE_LOAD off the critical path.
  * energies accumulate into ONE [8, 2560] PSUM tile (5 banks): the stationary
    for batch b is v (x) e_b, a [128, 8] one-hot column matrix, so batch b's
    matmuls land in PSUM row b while other batches' matmuls add exact zeros
    there.  Matmuls trail each tanh immediately; PE cost is free-size bound,
    unchanged by M=8.  The last batch's tanh is split so its matmuls (and the
    tail) start earlier.
  * softmax runs directly on [8, Tc]: ONE exp reads the whole PSUM row
    (PSUM->SBUF fused) with accum_out producing the row sums, then
    reciprocal, and the 1/rowsum scale splits ScalarE (activation Copy with
    per-partition scale) || DVE.  Out rows are fat 8704B packets on the sync
    queue.
"""

import sys

if "/opt/trn_rl_repo" not in sys.path:
    sys.path.insert(0, "/opt/trn_rl_repo")

import numpy as np

import concourse.bacc as bacc
import concourse.bass as bass
import concourse.tile as tile
from concourse import mybir
from concourse.bass_utils import run_bass_kernel_spmd

B, T, QD, AD = 64, 4096, 1024, 256
NCORES = 8
BLOC = B // NCORES  # batches per core
KB = QD // 128      # k-blocks for the pq matmul
DB = AD // 128      # d-blocks (partition blocks of AD)
F32 = mybir.dt.float32
F16 = mybir.dt.float16

# batches whose pm DMA rides the gpsimd hw-dynamic queue (the rest ride sync);
# gpsimd's DGE ramps up ~12us into the kernel, so early batches stay on sync
GP_BATCHES = (2, 4, 6)


def build_nc(Tc: int) -> bass.Bass:
    chunks = []
    lo = 0
    while lo < Tc:
        chunks.append((lo, min(512, Tc - lo)))
        lo += 512
    PSW = -(-Tc // 512) * 512  # psum tile width, whole banks

    nc = bacc.Bacc(None, target_bir_lowering=False)

    pm2 = nc.declare_dram_parameter("pm2", [BLOC, 128, DB, Tc], F16, isOutput=False)
    qT = nc.declare_dram_parameter("qT", [128, KB * BLOC], F16, isOutput=False)
    # WqR[p, kb, d] = Wq[d, kb*128 + p]  (host-packed: contiguous 512B rows)
    WqR = nc.declare_dram_parameter("WqR", [128, KB, AD], F16, isOutput=False)
    # v8[p, d*BLOC+b, j] = v[d*128+p] * (j == b): one-hot stationaries that
    # route batch b's energies into PSUM row b
    v8 = nc.declare_dram_parameter("v8", [128, DB * BLOC, 8], F16, isOutput=False)
    out = nc.declare_dram_parameter("out", [BLOC, Tc], F32, isOutput=True)

    Tanh = mybir.ActivationFunctionType.Tanh
    Exp = mybir.ActivationFunctionType.Exp
    Copy = mybir.ActivationFunctionType.Copy

    with tile.TileContext(nc) as tc:
        with (
            tc.tile_pool(name="singles", bufs=1) as singles,
            tc.tile_pool(name="pm", bufs=4) as pm_pool,
            tc.tile_pool(name="hid", bufs=3) as hid_pool,
            tc.tile_pool(name="energy", bufs=1, space="PSUM") as ep_pool,
            tc.tile_pool(name="spsum", bufs=2, space="PSUM") as sp_pool,
        ):
            # ---- act-table warm-up: dummy tanh so the 1.3us ACT_TABLE_LOAD
            # runs during startup DMA instead of gating the first real tanh
            zw = singles.tile([1, 1], F32)
            nc.gpsimd.memset(zw, 0)
            zw2 = singles.tile([1, 1], F32)
            nc.scalar.activation(out=zw2, in_=zw, func=Tanh, bias=zw[0:1, 0:1])

            # ---- constants on the sync queue, ahead of the pm stream ----
            wq_sb = singles.tile([128, KB, AD], F16)
            nc.sync.dma_start(out=wq_sb, in_=WqR[:, :, :])
            qt_sb = singles.tile([128, KB, BLOC], F16)
            nc.sync.dma_start(
                out=qt_sb, in_=qT[:, :].rearrange("p (kb b) -> p kb b", b=BLOC)
            )
            v8_sb = singles.tile([128, DB * BLOC, 8], F16)
            nc.sync.dma_start(out=v8_sb, in_=v8[:, :, :])

            # ---- pq = Wq @ query.T, laid out [d % 128, dblk, b] ----
            pq_sb = singles.tile([128, DB, BLOC], F32)
            for d in range(DB):
                ppq = sp_pool.tile([128, BLOC], F32, tag="sp")
                for k in range(KB):
                    nc.tensor.matmul(
                        ppq,
                        lhsT=wq_sb[:, k, d * 128 : (d + 1) * 128],
                        rhs=qt_sb[:, k, :],
                        start=(k == 0),
                        stop=(k == KB - 1),
                    )
                nc.vector.tensor_copy(out=pq_sb[:, d, :], in_=ppq)

            # ---- shared energies accumulator: one PSUM tile, 5 banks ----
            ep = ep_pool.tile([BLOC, PSW], F32)

            def emit_mms(b, d, lo_hi):
                first = b == 0 and d == 0
                last = b == BLOC - 1 and d == DB - 1
                for lo, w in chunks:
                    if lo < lo_hi[0] or lo >= lo_hi[1]:
                        continue
                    nc.tensor.matmul(
                        ep[:, lo : lo + w],
                        lhsT=v8_sb[:, d * BLOC + b, :],
                        rhs=h[:, d, lo : lo + w],
                        start=first,
                        stop=last,
                        skip_group_check=True,
                    )

            # ---- main loop: tanh + one-hot v-reduction ----
            for b in range(BLOC):
                pm_sb = pm_pool.tile([128, DB, Tc], F16)
                if b in GP_BATCHES:
                    nc.gpsimd.dma_start(out=pm_sb, in_=pm2[b])
                else:
                    nc.sync.dma_start(out=pm_sb, in_=pm2[b])
                h = hid_pool.tile([128, DB, Tc], F16)
                for d in range(DB):
                    last = b == BLOC - 1 and d == DB - 1
                    # split the very last tanh so its matmuls (and the whole
                    # softmax tail) start ~1.3us earlier
                    splits = [(0, 1536), (1536, Tc)] if last else [(0, Tc)]
                    for s0, s1 in splits:
                        nc.scalar.activation(
                            out=h[:, d, s0:s1],
                            in_=pm_sb[:, d, s0:s1],
                            func=Tanh,
                            bias=pq_sb[:, d, b : b + 1],
                            scale=1.0,
                        )
                        emit_mms(b, d, (s0, s1))

            # ---- softmax on [8, Tc]: one exp straight out of PSUM ----
            work = singles.tile([BLOC, Tc], F32)
            rsum = singles.tile([BLOC, 1], F32)
            nc.scalar.activation(
                out=work, in_=ep[:, 0:Tc], func=Exp, accum_out=rsum
            )
            rinv = singles.tile([BLOC, 1], F32)
            nc.vector.reciprocal(out=rinv, in_=rsum)

            # ---- scale by 1/rowsum, split ScalarE || DVE ----
            ow = singles.tile([BLOC, Tc], F32)
            SP = 1024  # ScalarE takes [0,SP), DVE takes [SP,Tc)
            nc.scalar.activation(
                out=ow[:, 0:SP], in_=work[:, 0:SP], func=Copy, scale=rinv[:, 0:1]
            )
            nc.vector.tensor_scalar_mul(
                out=ow[:, SP:Tc], in0=work[:, SP:Tc], scalar1=rinv[:, 0:1]
            )
            nc.sync.dma_start(out=out[:, :], in_=ow)

    nc.finalize()
    return nc


_CACHE: dict = {}


def _get_nc(key) -> bass.Bass:
    if key not in _CACHE:
        _CACHE[key] = build_nc(key)
    return _CACHE[key]


def _pick_tc(max_cnt: int) -> int:
    # fixed padded width, multiple of 128; 2176 covers the reference seed
    # (max count 2126) — recomputed per call so any mask works
    return max(2176, -(-(max_cnt + 1) // 128) * 128)


def make_in_maps(query, processed_memory, mask, Wq, v):
    query = np.ascontiguousarray(np.asarray(query, dtype=np.float32))
    pm = np.asarray(processed_memory, dtype=np.float32)
    mask_b = np.asarray(mask).astype(bool)
    Wq = np.asarray(Wq, dtype=np.float32)
    v = np.asarray(v, dtype=np.float32)

    keep = ~mask_b
    keep_idx = [np.flatnonzero(keep[gb]) for gb in range(B)]
    cnts = np.array([len(ix) for ix in keep_idx])
    Tc = _pick_tc(int(cnts.max()))
    key = Tc

    WqR = np.ascontiguousarray(
        Wq.T.reshape(KB, 128, AD).transpose(1, 0, 2).astype(np.float16)
    )
    v8 = np.zeros((128, DB * BLOC, 8), dtype=np.float16)
    for d in range(DB):
        for b in range(BLOC):
            v8[:, d * BLOC + b, b] = v[d * 128 : (d + 1) * 128]
    # padding fill: tanh(pq - 16*sign(v)) ~= -sign(v), so padded energies are
    # ~ -sum|v| ~= -12.8 -> exp ~ 3e-6: negligible in the row sum, and the
    # host scatter drops padded outputs entirely
    padfill = (-16.0 * np.sign(v)).astype(np.float16).reshape(DB, 128).T  # [128, DB]

    in_maps = []
    for i in range(NCORES):
        sl = slice(i * BLOC, (i + 1) * BLOC)
        pm2 = np.empty((BLOC, 128, DB, Tc), dtype=np.float16)
        pm2[:, :, :, :] = padfill[None, :, :, None]
        for b in range(BLOC):
            gb = i * BLOC + b
            c = cnts[gb]
            # [c, AD] -> [AD, c] -> [DB, 128, c] -> [128, DB, c]
            pm2[b, :, :, :c] = (
                pm[gb, keep_idx[gb], :].T.reshape(DB, 128, c).transpose(1, 0, 2)
            )
        in_maps.append(
            {
                "pm2": pm2,
                "qT": np.ascontiguousarray(
                    query[sl]
                    .T.reshape(KB, 128, BLOC)
                    .transpose(1, 0, 2)
                    .reshape(128, KB * BLOC)
                    .astype(np.float16)
                ),
                "WqR": WqR,
                "v8": v8,
            }
        )
    return in_maps, keep_idx, cnts, key


def run_spmd(in_maps, key=2176, **kwargs):
    return run_bass_kernel_spmd(_get_nc(key), in_maps, list(range(NCORES)), **kwargs)


def kernel(query, processed_memory, mask, Wq, v) -> np.ndarray:
    in_maps, keep_idx, cnts, key = make_in_maps(query, processed_memory, mask, Wq, v)
    res = run_spmd(in_maps, key=key)
    full = np.zeros((B, T), dtype=np.float32)
    for i in range(NCORES):
        outc = np.asarray(res.results[i]["out"], dtype=np.float32)
        for b in range(BLOC):
            gb = i * BLOC + b
            full[gb, keep_idx[gb]] = outc[b, : cnts[gb]]
    return full
